# revision 7
# baseline (speedup 1.0000x reference)
"""Trainium2 Bass kernel for the 2-layer LSTM language model.

Strategy: 8-way tensor parallelism over the hidden/gate dimension.
- Core k owns hidden slice k (128 of 1024 units) of both LSTM layers:
  it computes gate columns [f_k|i_k|o_k|t_k] (512 of 4096) each step.
- Per step, each core all-gathers its transposed h-slices so every core
  has the full h needed for the next step's contraction. Layer 1 lags
  layer 0 by one step so a single AllGather per step carries both
  [h0T_k(t) | h1T_k(t-1)].  The AllGather input is DMA'd straight from
  the transpose PSUM banks (no SBUF staging copy), and the output is
  received in two halves so the z0 matmuls unblock after the h0 half.
- The embedding, the speaker-flag rank-1 term, and b0 are folded into a
  single [384, 4096] input-side weight on the host, so the x-dependent
  part of z0 is 3 K=128 matmuls against a pre-transposed input stream.
  The x matmuls accumulate into PSUM before the AllGather lands.
- h1 for all steps is stored to DRAM; after the loop one big AllGather
  replicates it and each core computes the output MLP for its T/8 steps.
Matmul operands are bf16 (f32 PSUM accumulation); cell state stays f32.
"""
import numpy as np
import ml_dtypes

import concourse.bass as bass
import concourse.mybir as mybir
from concourse.bass_utils import run_bass_kernel_spmd

BF16 = ml_dtypes.bfloat16

T_FULL, B, IND = 512, 128, 259
EMB, NN, VOCAB, BIG = 512, 1024, 256, 128
NC = 8
SL = NN // NC          # 128 hidden units per core
GC = 4 * SL            # 512 gate columns per core
KP = 384               # padded inpT rows = 3 K-tiles (259 data + 1 + s + pad)
AF = mybir.dt.ActivationFunctionType if hasattr(mybir.dt, "ActivationFunctionType") else mybir.ActivationFunctionType
BF = mybir.dt.bfloat16
F32 = mybir.dt.float32


def build(T):
    TG = T // NC  # output steps per core
    nc = bass.Bass(target_bir_lowering=False, num_devices=NC)

    # ---- DRAM parameters (per core) ----
    inpT = nc.declare_dram_parameter("inpT", [KP, T * B], BF, isOutput=False)
    wc = nc.declare_dram_parameter("wc", [KP, GC], BF, isOutput=False)
    w0h = nc.declare_dram_parameter("w0h", [NN, GC], BF, isOutput=False)
    w1x = nc.declare_dram_parameter("w1x", [NN, GC], BF, isOutput=False)
    w1h = nc.declare_dram_parameter("w1h", [NN, GC], BF, isOutput=False)
    b1r = nc.declare_dram_parameter("b1r", [1, GC], BF, isOutput=False)
    ow0 = nc.declare_dram_parameter("ow0", [NN, NN], BF, isOutput=False)
    ob0c = nc.declare_dram_parameter("ob0c", [128, NC], F32, isOutput=False)
    ow1 = nc.declare_dram_parameter("ow1", [NN, VOCAB], BF, isOutput=False)
    ob1r = nc.declare_dram_parameter("ob1r", [1, VOCAB], BF, isOutput=False)
    iden = nc.declare_dram_parameter("iden", [128, 128], BF, isOutput=False)
    out = nc.declare_dram_parameter("out", [TG, B, VOCAB], F32, isOutput=True)

    # ---- internal DRAM ----
    ag_in = nc.dram_tensor("ag_in", [128, 2 * SL], BF)
    ag_out = nc.dram_tensor("ag_out", [NC * 128, 2 * SL], BF, addr_space="Shared")
    h1store = nc.dram_tensor("h1store", [128, T * SL], BF)
    ag2_out = nc.dram_tensor("ag2_out", [NC * 128, T * SL], BF, addr_space="Shared")

    # ---- SBUF ----
    wc_sb = nc.alloc_sbuf_tensor("wc_sb", [128, 3 * GC], BF)
    w0h_sb = nc.alloc_sbuf_tensor("w0h_sb", [128, 8 * GC], BF)
    w1x_sb = nc.alloc_sbuf_tensor("w1x_sb", [128, 8 * GC], BF)
    w1h_sb = nc.alloc_sbuf_tensor("w1h_sb", [128, 8 * GC], BF)
    b1_sb = nc.alloc_sbuf_tensor("b1_sb", [1, GC], BF)
    ones_sb = nc.alloc_sbuf_tensor("ones_sb", [1, 128], BF)
    id_sb = nc.alloc_sbuf_tensor("id_sb", [128, 128], BF)
    xb_sb = nc.alloc_sbuf_tensor("xb_sb", [128, 4 * 3 * 128], BF)  # 4 bufs x 3 tiles
    h0T_sb = nc.alloc_sbuf_tensor("h0T_sb", [128, 2 * NN], BF)  # 2 bufs x 8 tiles
    h1T_sb = nc.alloc_sbuf_tensor("h1T_sb", [128, 2 * NN], BF)
    g0_sb = nc.alloc_sbuf_tensor("g0_sb", [128, 3 * SL], F32)
    t0_sb = nc.alloc_sbuf_tensor("t0_sb", [128, SL], F32)
    th0_sb = nc.alloc_sbuf_tensor("th0_sb", [128, SL], F32)
    g1_sb = nc.alloc_sbuf_tensor("g1_sb", [128, 3 * SL], F32)
    t1_sb = nc.alloc_sbuf_tensor("t1_sb", [128, SL], F32)
    th1_sb = nc.alloc_sbuf_tensor("th1_sb", [128, SL], F32)
    c0_sb = nc.alloc_sbuf_tensor("c0_sb", [128, SL], F32)
    c1_sb = nc.alloc_sbuf_tensor("c1_sb", [128, SL], F32)
    tmpa_sb = nc.alloc_sbuf_tensor("tmpa_sb", [128, SL], F32)
    tmpb_sb = nc.alloc_sbuf_tensor("tmpb_sb", [128, SL], F32)
    h0l_sb = nc.alloc_sbuf_tensor("h0l_sb", [128, SL], BF)
    h1l_sb = nc.alloc_sbuf_tensor("h1l_sb", [128, SL], BF)
    send_sb = nc.alloc_sbuf_tensor("send_sb", [128, 2 * SL], BF)
    # output stage
    ow0_sb = nc.alloc_sbuf_tensor("ow0_sb", [128, 8 * NN], BF)
    ow1_sb = nc.alloc_sbuf_tensor("ow1_sb", [128, 8 * VOCAB], BF)
    ob0_sb = nc.alloc_sbuf_tensor("ob0_sb", [128, NC], F32)
    ob1_sb = nc.alloc_sbuf_tensor("ob1_sb", [1, VOCAB], BF)
    h14_sb = nc.alloc_sbuf_tensor("h14_sb", [128, 2 * 8 * 512], BF)  # 2 bufs
    hsT_sb = nc.alloc_sbuf_tensor("hsT_sb", [128, 2 * 8 * 512], BF)  # 2 bufs
    lg_sb = nc.alloc_sbuf_tensor("lg_sb", [128, 4 * VOCAB], F32)

    # ---- PSUM (8 banks total) ----
    psA = nc.alloc_psum_tensor("psA", [128, 512], F32)
    psB = nc.alloc_psum_tensor("psB", [128, 512], F32)
    psT0 = nc.alloc_psum_tensor("psT0", [128, 128], BF)
    psT1 = nc.alloc_psum_tensor("psT1", [128, 128], BF)
    hs_ps = [nc.alloc_psum_tensor(f"hs_ps{i}", [128, 512], F32) for i in range(4)]

    # ---- semaphores ----
    sems = {}
    for name in (
        "s_init", "s_vinit", "s_x", "s_xdone", "s_z0", "s_z1", "s_act0",
        "s_act1", "s_cmid0", "s_cmid1", "s_th0", "s_th1", "s_dve0", "s_dve1",
        "s_t0", "s_t1", "s_cp0", "s_cp1", "s_snd0", "s_snd1", "s_rcv0", "s_rcv1",
        "s_cc", "s_st",
        "s_og", "s_ohsT", "s_orelu", "s_olog", "s_ocp", "s_out",
    ):
        sems[name] = nc.alloc_semaphore(name)

    NGR = TG // 4  # output groups of 4 steps

    with nc.Block() as block:

        @block.sync
        def _(sync):
            n_init = 0

            def ld(dst, src):
                nonlocal n_init
                sync.dma_start(out=dst, in_=src).then_inc(sems["s_init"], 16)
                n_init += 1

            for j in range(3):
                ld(wc_sb[:, j * GC:(j + 1) * GC], wc[j * 128:(j + 1) * 128, :])
            for j in range(8):
                ld(w0h_sb[:, j * GC:(j + 1) * GC], w0h[j * 128:(j + 1) * 128, :])
                ld(w1x_sb[:, j * GC:(j + 1) * GC], w1x[j * 128:(j + 1) * 128, :])
                ld(w1h_sb[:, j * GC:(j + 1) * GC], w1h[j * 128:(j + 1) * 128, :])
                ld(ow0_sb[:, j * NN:(j + 1) * NN], ow0[j * 128:(j + 1) * 128, :])
                ld(ow1_sb[:, j * VOCAB:(j + 1) * VOCAB], ow1[j * 128:(j + 1) * 128, :])
            ld(b1_sb[:, :], b1r[:, :])
            ld(id_sb[:, :], iden[:, :])
            ld(ob0_sb[:, :], ob0c[:, :])
            ld(ob1_sb[:, :], ob1r[:, :])
            assert n_init == 47, n_init
            # x prefetch: inpT column block t -> xb slot t%4 (3 k-tiles)
            inpT3 = inpT.rearrange("(j p) n -> p j n", p=128)
            for t in range(T):
                if t >= 4:
                    sync.wait_ge(sems["s_xdone"], t - 3)
                s = (t % 4) * 3 * 128
                dst = xb_sb[:, s:s + 3 * 128].rearrange("p (j c) -> p j c", c=128)
                sync.dma_start(
                    out=dst, in_=inpT3[:, :, t * B:(t + 1) * B]
                ).then_inc(sems["s_x"], 16)

        @block.tensor
        def _(tensor):
            tensor.wait_ge(sems["s_init"], 16 * 47)
            tensor.wait_ge(sems["s_vinit"], 1)
            for t in range(T + 1):
                m = t - 1
                if t <= T - 1:
                    # z0(t) x-part into psA (before the AllGather lands)
                    tensor.wait_ge(sems["s_x"], 16 * (t + 1))
                    if t >= 1:
                        tensor.wait_ge(sems["s_act0"], t)
                    xoff = (t % 4) * 3 * 128
                    for j in range(3):
                        ins = tensor.matmul(
                            psA[:, :],
                            xb_sb[:, xoff + j * 128:xoff + (j + 1) * 128],
                            wc_sb[:, j * GC:(j + 1) * GC],
                            start=(j == 0),
                            stop=(t == 0 and j == 2),
                        )
                        if j == 2:
                            ins.then_inc(sems["s_xdone"], 1)
                    # z0(t) h-part
                    if t >= 1:
                        tensor.wait_ge(sems["s_rcv0"], 16 * t)
                        p = ((t - 1) % 2) * NN
                        for j in range(8):
                            ins = tensor.matmul(
                                psA[:, :],
                                h0T_sb[:, p + j * 128:p + (j + 1) * 128],
                                w0h_sb[:, j * GC:(j + 1) * GC],
                                start=False,
                                stop=(j == 7),
                            )
                            if j == 7:
                                ins.then_inc(sems["s_z0"], 1)
                if t >= 1:
                    # z1(m) into psB
                    if m >= 1:
                        tensor.wait_ge(sems["s_act1"], m)
                    if t == T:
                        tensor.wait_ge(sems["s_rcv0"], 16 * T)
                    tensor.wait_ge(sems["s_rcv1"], 16 * t)
                    tensor.matmul(psB[:, :], ones_sb[:, :], b1_sb[:, :], start=True, stop=False)
                    p = ((t - 1) % 2) * NN
                    for j in range(8):
                        tensor.matmul(
                            psB[:, :],
                            h0T_sb[:, p + j * 128:p + (j + 1) * 128],
                            w1x_sb[:, j * GC:(j + 1) * GC],
                            start=False, stop=False,
                        )
                    for j in range(8):
                        ins = tensor.matmul(
                            psB[:, :],
                            h1T_sb[:, p + j * 128:p + (j + 1) * 128],
                            w1h_sb[:, j * GC:(j + 1) * GC],
                            start=False, stop=(j == 7),
                        )
                        if j == 7:
                            ins.then_inc(sems["s_z1"], 1)
                if t <= T - 1:
                    # transpose h0l(t) -> psT0; WAR: snd0(t-1) must have read psT0
                    tensor.wait_ge(sems["s_dve0"], t + 1)
                    if t >= 1:
                        tensor.wait_ge(sems["s_cp0"], t)
                    tensor.transpose(psT0[:, 0:128], h0l_sb[:, :], id_sb[:, :]).then_inc(
                        sems["s_t0"], 1
                    )
                if t >= 1:
                    # transpose h1l(m) -> psT1; WAR: snd1(t-1) + h1store(t-1)
                    tensor.wait_ge(sems["s_dve1"], m + 1)
                    if m >= 1:
                        tensor.wait_ge(sems["s_cp1"], m + 1)
                    tensor.transpose(psT1[:, 0:128], h1l_sb[:, :], id_sb[:, :]).then_inc(
                        sems["s_t1"], 1
                    )
            # ---- output stage ----
            tensor.wait_ge(sems["s_cc"], T + 1)  # big AG done
            for g in range(NGR):
                tensor.wait_ge(sems["s_og"], 16 * 8 * (g + 1))
                hb = (g % 2) * 8 * 512
                if g >= 1:
                    tensor.wait_ge(sems["s_orelu"], 8 * g)
                    tensor.wait_ge(sems["s_ocp"], 4 * g)
                for mm in range(4):
                    for j in range(8):
                        ins = tensor.matmul(
                            hs_ps[mm][:, :],
                            ow0_sb[:, j * NN + mm * 128:j * NN + (mm + 1) * 128],
                            h14_sb[:, hb + j * 512:hb + (j + 1) * 512],
                            start=(j == 0), stop=(j == 7),
                        )
                        if j == 7:
                            ins.then_inc(sems["s_ohsT"], 1)
                tensor.wait_ge(sems["s_orelu"], 8 * g + 4)
                for mm in range(4, 8):
                    for j in range(8):
                        ins = tensor.matmul(
                            hs_ps[mm - 4][:, :],
                            ow0_sb[:, j * NN + mm * 128:j * NN + (mm + 1) * 128],
                            h14_sb[:, hb + j * 512:hb + (j + 1) * 512],
                            start=(j == 0), stop=(j == 7),
                        )
                        if j == 7:
                            ins.then_inc(sems["s_ohsT"], 1)
                tensor.wait_ge(sems["s_orelu"], 8 * (g + 1))
                ps_l = [psA, psB, hs_ps[0], hs_ps[1]]
                sb = (g % 2) * 8 * 512
                for tau in range(4):
                    tensor.matmul(
                        ps_l[tau][:, 0:VOCAB], ones_sb[:, :], ob1_sb[:, :],
                        start=True, stop=False,
                    )
                    for mm in range(8):
                        ins = tensor.matmul(
                            ps_l[tau][:, 0:VOCAB],
                            hsT_sb[:, sb + mm * 512 + tau * 128:sb + mm * 512 + (tau + 1) * 128],
                            ow1_sb[:, mm * VOCAB:(mm + 1) * VOCAB],
                            start=False, stop=(mm == 7),
                        )
                        if mm == 7:
                            ins.then_inc(sems["s_olog"], 1)

        @block.scalar
        def _(scalar):
            scalar.wait_ge(sems["s_init"], 16 * 47)
            SIG = AF.Sigmoid
            TANH = AF.Tanh
            for t in range(T + 1):
                m = t - 1
                if t <= T - 1:
                    if t == 0:
                        scalar.wait_ge(sems["s_xdone"], 1)
                    else:
                        scalar.wait_ge(sems["s_z0"], t)
                    if t >= 1:
                        scalar.wait_ge(sems["s_cmid0"], t)  # g0/t0 WAR
                    scalar.activation(g0_sb[:, :], psA[:, 0:3 * SL], SIG)
                    scalar.activation(t0_sb[:, :], psA[:, 3 * SL:4 * SL], TANH).then_inc(
                        sems["s_act0"], 1
                    )
                    scalar.wait_ge(sems["s_cmid0"], t + 1)
                    if t >= 1:
                        scalar.wait_ge(sems["s_dve0"], t)  # th0 WAR
                    scalar.activation(th0_sb[:, :], c0_sb[:, :], TANH).then_inc(
                        sems["s_th0"], 1
                    )
                if t >= 1:
                    scalar.wait_ge(sems["s_z1"], m + 1)
                    if m >= 1:
                        scalar.wait_ge(sems["s_cmid1"], m)
                    scalar.activation(g1_sb[:, :], psB[:, 0:3 * SL], SIG)
                    scalar.activation(t1_sb[:, :], psB[:, 3 * SL:4 * SL], TANH).then_inc(
                        sems["s_act1"], 1
                    )
                    scalar.wait_ge(sems["s_cmid1"], m + 1)
                    if m >= 1:
                        scalar.wait_ge(sems["s_dve1"], m)
                    scalar.activation(th1_sb[:, :], c1_sb[:, :], TANH).then_inc(
                        sems["s_th1"], 1
                    )
            # output: relu with per-partition bias
            for g in range(NGR):
                sb = (g % 2) * 8 * 512
                for mm in range(8):
                    scalar.wait_ge(sems["s_ohsT"], 8 * g + mm + 1)
                    if g >= 2:
                        scalar.wait_ge(sems["s_olog"], 4 * (g - 1))
                    scalar.activation(
                        hsT_sb[:, sb + mm * 512:sb + (mm + 1) * 512],
                        hs_ps[mm % 4][:, :],
                        AF.Relu,
                        bias=ob0_sb[:, mm:mm + 1],
                    ).then_inc(sems["s_orelu"], 1)

        @block.vector
        def _(vector):
            vector.memset(send_sb[:, :], 0.0).then_inc(sems["s_cp1"], 1)
            vector.memset(c0_sb[:, :], 0.0)
            vector.memset(c1_sb[:, :], 0.0)
            vector.memset(ones_sb[:, :], 1.0).then_inc(sems["s_vinit"], 1)
            MUL = mybir.AluOpType.mult
            for t in range(T + 1):
                m = t - 1
                if t <= T - 1:
                    vector.wait_ge(sems["s_act0"], t + 1)
                    vector.tensor_tensor(tmpa_sb[:, :], g0_sb[:, 0:SL], c0_sb[:, :], MUL)
                    vector.tensor_tensor(tmpb_sb[:, :], g0_sb[:, SL:2 * SL], t0_sb[:, :], MUL)
                    vector.tensor_add(c0_sb[:, :], tmpa_sb[:, :], tmpb_sb[:, :]).then_inc(
                        sems["s_cmid0"], 1
                    )
                    vector.wait_ge(sems["s_th0"], t + 1)
                    vector.tensor_tensor(
                        h0l_sb[:, :], g0_sb[:, 2 * SL:3 * SL], th0_sb[:, :], MUL
                    ).then_inc(sems["s_dve0"], 1)
                    vector.wait_ge(sems["s_t0"], t + 1)
                    if t >= 1:
                        vector.wait_ge(sems["s_snd0"], 16 * t)
                    vector.tensor_copy(send_sb[:, 0:SL], psT0[:, 0:128]).then_inc(
                        sems["s_cp0"], 1
                    )
                if t >= 1:
                    vector.wait_ge(sems["s_act1"], m + 1)
                    vector.tensor_tensor(tmpa_sb[:, :], g1_sb[:, 0:SL], c1_sb[:, :], MUL)
                    vector.tensor_tensor(tmpb_sb[:, :], g1_sb[:, SL:2 * SL], t1_sb[:, :], MUL)
                    vector.tensor_add(c1_sb[:, :], tmpa_sb[:, :], tmpb_sb[:, :]).then_inc(
                        sems["s_cmid1"], 1
                    )
                    vector.wait_ge(sems["s_th1"], m + 1)
                    vector.tensor_tensor(
                        h1l_sb[:, :], g1_sb[:, 2 * SL:3 * SL], th1_sb[:, :], MUL
                    ).then_inc(sems["s_dve1"], 1)
                    vector.wait_ge(sems["s_t1"], m + 1)
                    vector.wait_ge(sems["s_snd1"], 16 * t)
                    if m >= 1:
                        vector.wait_ge(sems["s_st"], 16 * m)
                    vector.tensor_copy(send_sb[:, SL:2 * SL], psT1[:, 0:128]).then_inc(
                        sems["s_cp1"], 1
                    )
            # output: copy logits psum -> sbuf
            ps_l = [psA, psB, hs_ps[0], hs_ps[1]]
            for g in range(NGR):
                for tau in range(4):
                    vector.wait_ge(sems["s_olog"], 4 * g + tau + 1)
                    if g >= 1:
                        vector.wait_ge(sems["s_out"], 16 * (4 * (g - 1) + tau + 1))
                    vector.tensor_copy(
                        lg_sb[:, tau * VOCAB:(tau + 1) * VOCAB], ps_l[tau][:, 0:VOCAB]
                    ).then_inc(sems["s_ocp"], 1)

        @block.gpsimd
        def _(gpsimd):
            ag_out3 = ag_out.rearrange("(j p) c -> p j c", p=128)
            for t in range(T + 1):
                m = t - 1
                if t <= T - 1:
                    # send: send_sb halves -> ag_in
                    gpsimd.wait_ge(sems["s_cp0"], t + 1)
                    if t >= 1:
                        gpsimd.wait_ge(sems["s_cc"], t)  # ag_in free (AG t-1 done)
                    gpsimd.dma_start(
                        out=ag_in[:, 0:SL], in_=send_sb[:, 0:SL]
                    ).then_inc(sems["s_snd0"], 16)
                    gpsimd.wait_ge(sems["s_cp1"], t + 1)
                    gpsimd.dma_start(
                        out=ag_in[:, SL:2 * SL], in_=send_sb[:, SL:2 * SL]
                    ).then_inc(sems["s_snd1"], 16)
                    # h1store <- send_sb h1 half (h1(t-1))
                    if t >= 1:
                        gpsimd.dma_start(
                            out=h1store[:, m * SL:(m + 1) * SL],
                            in_=send_sb[:, SL:2 * SL],
                        ).then_inc(sems["s_st"], 16)
                    gpsimd.wait_ge(sems["s_snd0"], 16 * (t + 1))
                    gpsimd.wait_ge(sems["s_snd1"], 16 * (t + 1))
                    gpsimd.collective_compute(
                        "AllGather",
                        mybir.AluOpType.bypass,
                        replica_groups=[list(range(NC))],
                        ins=[ag_in.ap().opt()],
                        outs=[ag_out.ap().opt()],
                    ).then_inc(sems["s_cc"], 1)
                    gpsimd.wait_ge(sems["s_cc"], t + 1)
                    if t >= 2:
                        gpsimd.wait_ge(sems["s_z1"], t - 1)  # WAR on h buffers
                    p = (t % 2) * NN
                    gpsimd.dma_start(
                        out=h0T_sb[:, p:p + NN].rearrange("p (j c) -> p j c", c=128),
                        in_=ag_out3[:, :, 0:SL],
                    ).then_inc(sems["s_rcv0"], 16)
                    gpsimd.dma_start(
                        out=h1T_sb[:, p:p + NN].rearrange("p (j c) -> p j c", c=128),
                        in_=ag_out3[:, :, SL:2 * SL],
                    ).then_inc(sems["s_rcv1"], 16)
                else:  # t == T: store last h1 slice only
                    gpsimd.wait_ge(sems["s_cp1"], T + 1)
                    gpsimd.dma_start(
                        out=h1store[:, m * SL:(m + 1) * SL],
                        in_=send_sb[:, SL:2 * SL],
                    ).then_inc(sems["s_st"], 16)
            # big AllGather of the h1 series
            gpsimd.wait_ge(sems["s_st"], 16 * T)
            gpsimd.collective_compute(
                "AllGather",
                mybir.AluOpType.bypass,
                replica_groups=[list(range(NC))],
                ins=[h1store.ap().opt()],
                outs=[ag2_out.ap().opt()],
            ).then_inc(sems["s_cc"], 1)
            gpsimd.wait_ge(sems["s_cc"], T + 1)
            # output stage: loads + stores
            rank = gpsimd.partition_id()
            rreg = gpsimd.to_reg(rank)
            for g in range(NGR):
                if g >= 2:
                    gpsimd.wait_ge(sems["s_ohsT"], 8 * (g - 1))
                hb = (g % 2) * 8 * 512
                for k in range(NC):
                    with gpsimd.If_eq(rreg, k):
                        cb = k * TG * SL + g * 4 * SL
                        for j in range(8):
                            gpsimd.dma_start(
                                out=h14_sb[:, hb + j * 512:hb + (j + 1) * 512],
                                in_=ag2_out[128 * j:128 * (j + 1), cb:cb + 4 * SL],
                            ).then_inc(sems["s_og"], 16)
                if g >= 1:
                    for tau in range(4):
                        gpsimd.wait_ge(sems["s_ocp"], 4 * (g - 1) + tau + 1)
                        gpsimd.dma_start(
                            out=out[4 * (g - 1) + tau, :, :],
                            in_=lg_sb[:, tau * VOCAB:(tau + 1) * VOCAB],
                        ).then_inc(sems["s_out"], 16)
            g = NGR
            for tau in range(4):
                gpsimd.wait_ge(sems["s_ocp"], 4 * (g - 1) + tau + 1)
                gpsimd.dma_start(
                    out=out[4 * (g - 1) + tau, :, :],
                    in_=lg_sb[:, tau * VOCAB:(tau + 1) * VOCAB],
                ).then_inc(sems["s_out"], 16)

    return nc


def _host_prep(inputs, T):
    inp = np.ascontiguousarray(inputs["inputs"][:T]).astype(np.float32)
    emb_W = inputs["emb_W"].astype(np.float32)
    W0 = inputs["lstm_W0"].astype(np.float32)
    b0 = inputs["lstm_b0"].astype(np.float32)
    W1 = inputs["lstm_W1"].astype(np.float32)
    b1 = inputs["lstm_b1"].astype(np.float32)

    flat = inp.reshape(T * B, IND)
    s = np.where(
        (flat[:, VOCAB] == 1.0) & (flat[:, VOCAB + 1] == 0.0), 1.0, -1.0
    ).astype(np.float32)
    inpT_aug = np.zeros((KP, T * B), np.float32)
    inpT_aug[:IND] = flat.T
    inpT_aug[IND] = 1.0
    inpT_aug[IND + 1] = s

    # x-side folded weight: emb @ W0[:512] + flags(b0 row) + rank1(u row)
    Wc = np.zeros((KP, 4 * NN), np.float32)
    Wc[:IND] = emb_W @ W0[:EMB]
    Wc[IND] = b0
    Wc[IND + 1] = W0[EMB:EMB + BIG].sum(axis=0)

    W0h = W0[EMB + BIG:]            # [1024, 4096]
    W1x, W1h = W1[:NN], W1[NN:]

    def gate_cols(W, k):
        return np.concatenate(
            [W[:, base + k * SL:base + (k + 1) * SL] for base in
             (0, NN, 2 * NN, 3 * NN)], axis=1)

    bf = lambda x: np.ascontiguousarray(x).astype(BF16)
    inpT_bf = bf(inpT_aug)
    ow0 = bf(inputs["out_W0"])
    ob0c = np.ascontiguousarray(
        inputs["out_b0"].astype(np.float32).reshape(NC, 128).T
    )
    ow1 = bf(inputs["out_W1"])
    ob1r = bf(inputs["out_b1"].reshape(1, VOCAB))
    iden = bf(np.eye(128, dtype=np.float32))

    in_maps = []
    for k in range(NC):
        in_maps.append({
            "inpT": inpT_bf,
            "wc": bf(gate_cols(Wc, k)),
            "w0h": bf(gate_cols(W0h, k)),
            "w1x": bf(gate_cols(W1x, k)),
            "w1h": bf(gate_cols(W1h, k)),
            "b1r": bf(gate_cols(b1.reshape(1, 4 * NN), k)),
            "ow0": ow0,
            "ob0c": ob0c,
            "ow1": ow1,
            "ob1r": ob1r,
            "iden": iden,
        })
    return in_maps


_CACHE = {}


def run(inputs, T=T_FULL, trace=False):
    if T not in _CACHE:
        _CACHE[T] = build(T)
    nc = _CACHE[T]
    in_maps = _host_prep(inputs, T)
    res = run_bass_kernel_spmd(
        nc, in_maps, core_ids=list(range(NC)), trace=trace
    )
    out = np.concatenate([res.results[k]["out"] for k in range(NC)], axis=0)
    return out, res


def kernel(**inputs):
    out, _ = run(inputs, T=T_FULL)
    return out.astype(np.float32)


# revision 10
# speedup vs baseline: 1.1100x; 1.1100x over previous
"""Trainium2 Bass kernel for the 2-layer LSTM language model.

Strategy: 8-way tensor parallelism over the hidden/gate dimension.
- Core k owns hidden slice k (128 of 1024 units) of both LSTM layers:
  it computes gate columns [f_k|i_k|o_k|t_k] (512 of 4096) each step.
- Per step one AllGather distributes [h0T_k(t) | h1T_k(t-2)].  Layer 1
  lags layer 0 by TWO steps so the AllGather doorbell waits only on the
  (critical) layer-0 chain; the layer-1 half was transposed a full
  iteration earlier.  z1(m) runs at iteration m+2 and consumes
  h0T(m) / h1T(m-1) exactly as the math requires - only the wall-clock
  schedule shifts, not the values.
- The embedding, the speaker-flag rank-1 term, and b0 are folded into a
  single [384, 4096] input-side weight on the host; the x-part of z0
  accumulates into PSUM before the AllGather lands.
- The output MLP (relu(h1@ow0+b0)@ow1+b1) is computed INLINE: every
  step, the h1 tiles already distributed by the AllGather are DMA'd
  from ag_out into a deep SBUF ring (mlpbuf); groups of 4 steps are
  processed round-robin (group g -> core g%8) as small per-iteration
  chunks that fill tensor-engine idle time under the AllGather.  No
  h1 DRAM store, no trailing big AllGather, no output tail.
Matmul operands are bf16 (f32 PSUM accumulation); cell state stays f32.
"""
import numpy as np
import ml_dtypes

import concourse.bass as bass
import concourse.mybir as mybir
from concourse.bass_utils import run_bass_kernel_spmd

BF16 = ml_dtypes.bfloat16

T_FULL, B, IND = 512, 128, 259
EMB, NN, VOCAB, BIG = 512, 1024, 256, 128
NC = 8
SL = NN // NC          # 128 hidden units per core
GC = 4 * SL            # 512 gate columns per core
KP = 384               # padded inpT rows = 3 K-tiles (259 data + 1 + s + pad)
MDEPTH = 8             # mlpbuf ring depth in groups (8 groups = 32 steps)
AF = mybir.dt.ActivationFunctionType if hasattr(mybir.dt, "ActivationFunctionType") else mybir.ActivationFunctionType
BF = mybir.dt.bfloat16
F32 = mybir.dt.float32


def build(T):
    NG = T // 4            # output groups of 4 steps (round-robin over cores)
    TG = 4 * (NG // NC)    # output rows per core
    assert NG % NC == 0
    nc = bass.Bass(target_bir_lowering=False, num_devices=NC)

    # ---- DRAM parameters (per core) ----
    inpT = nc.declare_dram_parameter("inpT", [KP, T * B], BF, isOutput=False)
    wc = nc.declare_dram_parameter("wc", [KP, GC], BF, isOutput=False)
    w0h = nc.declare_dram_parameter("w0h", [NN, GC], BF, isOutput=False)
    w1x = nc.declare_dram_parameter("w1x", [NN, GC], BF, isOutput=False)
    w1h = nc.declare_dram_parameter("w1h", [NN, GC], BF, isOutput=False)
    b1r = nc.declare_dram_parameter("b1r", [1, GC], BF, isOutput=False)
    ow0 = nc.declare_dram_parameter("ow0", [NN, NN], BF, isOutput=False)
    ob0c = nc.declare_dram_parameter("ob0c", [128, NC], F32, isOutput=False)
    ow1 = nc.declare_dram_parameter("ow1", [NN, VOCAB], BF, isOutput=False)
    ob1r = nc.declare_dram_parameter("ob1r", [1, VOCAB], BF, isOutput=False)
    iden = nc.declare_dram_parameter("iden", [128, 128], BF, isOutput=False)
    out = nc.declare_dram_parameter("out", [TG, B, VOCAB], F32, isOutput=True)

    # ---- internal DRAM ----
    ag_in = nc.dram_tensor("ag_in", [128, 2 * SL], BF)
    ag_out = nc.dram_tensor("ag_out", [NC * 128, 2 * SL], BF, addr_space="Shared")

    # ---- SBUF ----
    wc_sb = nc.alloc_sbuf_tensor("wc_sb", [128, 3 * GC], BF)
    w0h_sb = nc.alloc_sbuf_tensor("w0h_sb", [128, 8 * GC], BF)
    w1x_sb = nc.alloc_sbuf_tensor("w1x_sb", [128, 8 * GC], BF)
    w1h_sb = nc.alloc_sbuf_tensor("w1h_sb", [128, 8 * GC], BF)
    b1_sb = nc.alloc_sbuf_tensor("b1_sb", [1, GC], BF)
    ones_sb = nc.alloc_sbuf_tensor("ones_sb", [1, 128], BF)
    id_sb = nc.alloc_sbuf_tensor("id_sb", [128, 128], BF)
    xb_sb = nc.alloc_sbuf_tensor("xb_sb", [128, 4 * 3 * 128], BF)  # 4 bufs x 3 tiles
    h0T_sb = nc.alloc_sbuf_tensor("h0T_sb", [128, 2 * NN], BF)  # 2 bufs x 8 tiles
    h1T_sb = nc.alloc_sbuf_tensor("h1T_sb", [128, 2 * NN], BF)
    g0_sb = nc.alloc_sbuf_tensor("g0_sb", [128, 3 * SL], F32)
    t0_sb = nc.alloc_sbuf_tensor("t0_sb", [128, SL], F32)
    th0_sb = nc.alloc_sbuf_tensor("th0_sb", [128, SL], F32)
    g1_sb = nc.alloc_sbuf_tensor("g1_sb", [128, 3 * SL], F32)
    t1_sb = nc.alloc_sbuf_tensor("t1_sb", [128, SL], F32)
    th1_sb = nc.alloc_sbuf_tensor("th1_sb", [128, SL], F32)
    c0_sb = nc.alloc_sbuf_tensor("c0_sb", [128, SL], F32)
    c1_sb = nc.alloc_sbuf_tensor("c1_sb", [128, SL], F32)
    tmpa_sb = nc.alloc_sbuf_tensor("tmpa_sb", [128, SL], F32)
    tmpb_sb = nc.alloc_sbuf_tensor("tmpb_sb", [128, SL], F32)
    h0l_sb = nc.alloc_sbuf_tensor("h0l_sb", [128, SL], BF)
    h1l_sb = nc.alloc_sbuf_tensor("h1l_sb", [128, SL], BF)
    send_sb = nc.alloc_sbuf_tensor("send_sb", [128, 2 * SL], BF)
    # inline output MLP
    mlpbuf = nc.alloc_sbuf_tensor("mlpbuf", [128, MDEPTH * 8 * 4 * 128], BF)
    ow0_sb = nc.alloc_sbuf_tensor("ow0_sb", [128, 8 * NN], BF)
    ow1_sb = nc.alloc_sbuf_tensor("ow1_sb", [128, 8 * VOCAB], BF)
    ob0_sb = nc.alloc_sbuf_tensor("ob0_sb", [128, NC], F32)
    ob1_sb = nc.alloc_sbuf_tensor("ob1_sb", [1, VOCAB], BF)
    hsT_sb = nc.alloc_sbuf_tensor("hsT_sb", [128, 8 * 512], BF)
    lg_sb = nc.alloc_sbuf_tensor("lg_sb", [128, 4 * VOCAB], F32)

    # ---- PSUM (7 of 8 banks) ----
    psA = nc.alloc_psum_tensor("psA", [128, 512], F32)
    psB = nc.alloc_psum_tensor("psB", [128, 512], F32)
    psT0 = nc.alloc_psum_tensor("psT0", [128, 128], BF)
    psT1 = nc.alloc_psum_tensor("psT1", [128, 128], BF)
    hs_ps = [nc.alloc_psum_tensor(f"hs_ps{i}", [128, 512], F32) for i in range(3)]

    # ---- semaphores ----
    sems = {}
    for name in (
        "s_init", "s_vinit", "s_x", "s_xdone", "s_z0", "s_z1", "s_act0",
        "s_act1", "s_cmid0", "s_cmid1", "s_th0", "s_th1", "s_dve0", "s_dve1",
        "s_t0", "s_t1", "s_cp0", "s_cp1", "s_snd0", "s_snd1", "s_rcv0",
        "s_rcv1", "s_cc", "s_mdma",
        "s_ohsT", "s_orelu", "s_olog", "s_ocp", "s_out",
    ):
        sems[name] = nc.alloc_semaphore(name)

    # ---- inline-MLP chunk schedule ----
    # group g covers steps 4g..4g+3, owner core g%8, data complete after
    # gpsimd step 4g+5 (s_mdma >= 16*(4g+4)).  Chunks at iteration
    # tau = 4g+6+d: d=0..7 -> hs m-tile d; d=8..11 -> logits step d-8.
    # Owner-local group ordinal og = g//8.
    chunk_at = {}
    for g in range(NG):
        for d in range(12):
            chunk_at.setdefault(4 * g + 6 + d, []).append((g, d))
    outdma_at = {}
    for g in range(NG):
        outdma_at.setdefault(4 * g + 18, []).append(g)
    TEND = max(T + 2, max(chunk_at) + 1)
    GEND = max(T + 2, max(outdma_at) + 1)

    q4 = 8 * 4 * 128  # mlpbuf columns per group slot

    with nc.Block() as block:

        @block.sync
        def _(sync):
            n_init = 0

            def ld(dst, src):
                nonlocal n_init
                sync.dma_start(out=dst, in_=src).then_inc(sems["s_init"], 16)
                n_init += 1

            for j in range(3):
                ld(wc_sb[:, j * GC:(j + 1) * GC], wc[j * 128:(j + 1) * 128, :])
            for j in range(8):
                ld(w0h_sb[:, j * GC:(j + 1) * GC], w0h[j * 128:(j + 1) * 128, :])
                ld(w1x_sb[:, j * GC:(j + 1) * GC], w1x[j * 128:(j + 1) * 128, :])
                ld(w1h_sb[:, j * GC:(j + 1) * GC], w1h[j * 128:(j + 1) * 128, :])
                ld(ow0_sb[:, j * NN:(j + 1) * NN], ow0[j * 128:(j + 1) * 128, :])
                ld(ow1_sb[:, j * VOCAB:(j + 1) * VOCAB], ow1[j * 128:(j + 1) * 128, :])
            ld(b1_sb[:, :], b1r[:, :])
            ld(id_sb[:, :], iden[:, :])
            ld(ob0_sb[:, :], ob0c[:, :])
            ld(ob1_sb[:, :], ob1r[:, :])
            assert n_init == 47, n_init
            # x prefetch: inpT column block t -> xb slot t%4 (3 k-tiles)
            inpT3 = inpT.rearrange("(j p) n -> p j n", p=128)
            for t in range(T):
                if t >= 4:
                    sync.wait_ge(sems["s_xdone"], t - 3)
                s = (t % 4) * 3 * 128
                dst = xb_sb[:, s:s + 3 * 128].rearrange("p (j c) -> p j c", c=128)
                sync.dma_start(
                    out=dst, in_=inpT3[:, :, t * B:(t + 1) * B]
                ).then_inc(sems["s_x"], 16)

        @block.tensor
        def _(tensor):
            tensor.wait_ge(sems["s_init"], 16 * 47)
            tensor.wait_ge(sems["s_vinit"], 1)
            rreg = tensor.to_reg(tensor.partition_id())
            for t in range(TEND):
                m2 = t - 2
                if 2 <= t <= T + 1:
                    # z1(m2) bias + w1x part first: h0T(m2) came from an
                    # older AllGather, so this fills the current AG wait.
                    if m2 >= 1:
                        tensor.wait_ge(sems["s_act1"], m2)
                    if t >= T:
                        tensor.wait_ge(sems["s_rcv0"], 16 * T)
                    tensor.matmul(psB[:, :], ones_sb[:, :], b1_sb[:, :], start=True, stop=False)
                    p0 = (m2 % 2) * NN
                    for j in range(8):
                        tensor.matmul(
                            psB[:, :],
                            h0T_sb[:, p0 + j * 128:p0 + (j + 1) * 128],
                            w1x_sb[:, j * GC:(j + 1) * GC],
                            start=False, stop=False,
                        )
                if t <= T - 1:
                    # z0(t) x-part into psA (before the AllGather lands)
                    tensor.wait_ge(sems["s_x"], 16 * (t + 1))
                    if t >= 1:
                        tensor.wait_ge(sems["s_act0"], t)
                    xoff = (t % 4) * 3 * 128
                    for j in range(3):
                        ins = tensor.matmul(
                            psA[:, :],
                            xb_sb[:, xoff + j * 128:xoff + (j + 1) * 128],
                            wc_sb[:, j * GC:(j + 1) * GC],
                            start=(j == 0),
                            stop=(t == 0 and j == 2),
                        )
                        if j == 2:
                            ins.then_inc(sems["s_xdone"], 1)
                    # z0(t) h-part
                    if t >= 1:
                        tensor.wait_ge(sems["s_rcv0"], 16 * t)
                        p = ((t - 1) % 2) * NN
                        for j in range(8):
                            ins = tensor.matmul(
                                psA[:, :],
                                h0T_sb[:, p + j * 128:p + (j + 1) * 128],
                                w0h_sb[:, j * GC:(j + 1) * GC],
                                start=False,
                                stop=(j == 7),
                            )
                            if j == 7:
                                ins.then_inc(sems["s_z0"], 1)
                if 2 <= t <= T + 1:
                    # z1(m2) w1h part: h1T(m2-1) arrives with the same AG as
                    # h0T(t-1), right before this point.
                    tensor.wait_ge(sems["s_rcv1"], 16 * t)
                    p1 = ((m2 + 1) % 2) * NN
                    for j in range(8):
                        ins = tensor.matmul(
                            psB[:, :],
                            h1T_sb[:, p1 + j * 128:p1 + (j + 1) * 128],
                            w1h_sb[:, j * GC:(j + 1) * GC],
                            start=False, stop=(j == 7),
                        )
                        if j == 7:
                            ins.then_inc(sems["s_z1"], 1)
                if t <= T - 1:
                    # transpose h0l(t) -> psT0; WAR: cp0(t-1) read psT0
                    tensor.wait_ge(sems["s_dve0"], t + 1)
                    if t >= 1:
                        tensor.wait_ge(sems["s_cp0"], t)
                    tensor.transpose(psT0[:, 0:128], h0l_sb[:, :], id_sb[:, :]).then_inc(
                        sems["s_t0"], 1
                    )
                if 2 <= t <= T + 1:
                    # transpose h1l(m2) -> psT1; WAR: cp1(t-1) read psT1
                    tensor.wait_ge(sems["s_dve1"], t - 1)
                    if t >= 3:
                        tensor.wait_ge(sems["s_cp1"], t - 1)
                    tensor.transpose(psT1[:, 0:128], h1l_sb[:, :], id_sb[:, :]).then_inc(
                        sems["s_t1"], 1
                    )
                # ---- inline MLP chunks ----
                for (g, d) in chunk_at.get(t, ()):
                    k = g % NC
                    og = g // NC
                    with tensor.If_eq(rreg, k):
                        if d == 0:
                            tensor.wait_ge(sems["s_mdma"], 16 * (4 * g + 4))
                        if d <= 7:
                            # hs m-tile d -> hs_ps[d%2]
                            if og * 8 + d >= 2:
                                tensor.wait_ge(sems["s_orelu"], 8 * og + d - 1)
                            mb = (g % MDEPTH) * q4
                            for j in range(8):
                                ins = tensor.matmul(
                                    hs_ps[d % 2][:, :],
                                    ow0_sb[:, j * NN + d * 128:j * NN + (d + 1) * 128],
                                    mlpbuf[:, mb + j * 512:mb + (j + 1) * 512],
                                    start=(j == 0), stop=(j == 7),
                                )
                                if j == 7:
                                    ins.then_inc(sems["s_ohsT"], 1)
                        else:
                            i = d - 8
                            # logits for step 4g+i -> hs_ps[2]
                            if d == 8:
                                tensor.wait_ge(sems["s_orelu"], 8 * (og + 1))
                            if og * 4 + i >= 1:
                                tensor.wait_ge(sems["s_ocp"], 4 * og + i)
                            tensor.matmul(
                                hs_ps[2][:, 0:VOCAB], ones_sb[:, :], ob1_sb[:, :],
                                start=True, stop=False,
                            )
                            for mm in range(8):
                                ins = tensor.matmul(
                                    hs_ps[2][:, 0:VOCAB],
                                    hsT_sb[:, mm * 512 + i * 128:mm * 512 + (i + 1) * 128],
                                    ow1_sb[:, mm * VOCAB:(mm + 1) * VOCAB],
                                    start=False, stop=(mm == 7),
                                )
                                if mm == 7:
                                    ins.then_inc(sems["s_olog"], 1)

        @block.scalar
        def _(scalar):
            scalar.wait_ge(sems["s_init"], 16 * 47)
            rreg = scalar.to_reg(scalar.partition_id())
            SIG = AF.Sigmoid
            TANH = AF.Tanh
            for t in range(TEND):
                m2 = t - 2
                if t <= T - 1:
                    if t == 0:
                        scalar.wait_ge(sems["s_xdone"], 1)
                    else:
                        scalar.wait_ge(sems["s_z0"], t)
                    if t >= 1:
                        scalar.wait_ge(sems["s_cmid0"], t)  # g0/t0 WAR
                    scalar.activation(g0_sb[:, :], psA[:, 0:3 * SL], SIG)
                    scalar.activation(t0_sb[:, :], psA[:, 3 * SL:4 * SL], TANH).then_inc(
                        sems["s_act0"], 1
                    )
                if 2 <= t <= T + 1:
                    scalar.wait_ge(sems["s_z1"], m2 + 1)
                    if m2 >= 1:
                        scalar.wait_ge(sems["s_cmid1"], m2)
                    scalar.activation(g1_sb[:, :], psB[:, 0:3 * SL], SIG)
                    scalar.activation(t1_sb[:, :], psB[:, 3 * SL:4 * SL], TANH).then_inc(
                        sems["s_act1"], 1
                    )
                if t <= T - 1:
                    scalar.wait_ge(sems["s_cmid0"], t + 1)
                    if t >= 1:
                        scalar.wait_ge(sems["s_dve0"], t)  # th0 WAR
                    scalar.activation(th0_sb[:, :], c0_sb[:, :], TANH).then_inc(
                        sems["s_th0"], 1
                    )
                if 2 <= t <= T + 1:
                    scalar.wait_ge(sems["s_cmid1"], m2 + 1)
                    if m2 >= 1:
                        scalar.wait_ge(sems["s_dve1"], m2)
                    scalar.activation(th1_sb[:, :], c1_sb[:, :], TANH).then_inc(
                        sems["s_th1"], 1
                    )
                # inline MLP: relu for hs m-tiles finished at this iteration
                for (g, d) in chunk_at.get(t, ()):
                    if d > 7:
                        continue
                    k = g % NC
                    og = g // NC
                    with scalar.If_eq(rreg, k):
                        scalar.wait_ge(sems["s_ohsT"], 8 * og + d + 1)
                        if og >= 1 and d <= 1:
                            scalar.wait_ge(sems["s_olog"], 4 * og)
                        scalar.activation(
                            hsT_sb[:, d * 512:(d + 1) * 512],
                            hs_ps[d % 2][:, :],
                            AF.Relu,
                            bias=ob0_sb[:, d:d + 1],
                        ).then_inc(sems["s_orelu"], 1)

        @block.vector
        def _(vector):
            vector.memset(send_sb[:, :], 0.0).then_inc(sems["s_cp1"], 1)
            vector.memset(c0_sb[:, :], 0.0)
            vector.memset(c1_sb[:, :], 0.0)
            vector.memset(ones_sb[:, :], 1.0).then_inc(sems["s_vinit"], 1)
            rreg = vector.to_reg(vector.partition_id())
            MUL = mybir.AluOpType.mult
            for t in range(TEND):
                m2 = t - 2
                if t <= T - 1:
                    vector.wait_ge(sems["s_act0"], t + 1)
                    vector.tensor_tensor(tmpa_sb[:, :], g0_sb[:, 0:SL], c0_sb[:, :], MUL)
                    vector.tensor_tensor(tmpb_sb[:, :], g0_sb[:, SL:2 * SL], t0_sb[:, :], MUL)
                    vector.tensor_add(c0_sb[:, :], tmpa_sb[:, :], tmpb_sb[:, :]).then_inc(
                        sems["s_cmid0"], 1
                    )
                if 2 <= t <= T + 1:
                    vector.wait_ge(sems["s_act1"], m2 + 1)
                    vector.tensor_tensor(tmpa_sb[:, :], g1_sb[:, 0:SL], c1_sb[:, :], MUL)
                    vector.tensor_tensor(tmpb_sb[:, :], g1_sb[:, SL:2 * SL], t1_sb[:, :], MUL)
                    vector.tensor_add(c1_sb[:, :], tmpa_sb[:, :], tmpb_sb[:, :]).then_inc(
                        sems["s_cmid1"], 1
                    )
                if t <= T - 1:
                    vector.wait_ge(sems["s_th0"], t + 1)
                    vector.tensor_tensor(
                        h0l_sb[:, :], g0_sb[:, 2 * SL:3 * SL], th0_sb[:, :], MUL
                    ).then_inc(sems["s_dve0"], 1)
                    vector.wait_ge(sems["s_t0"], t + 1)
                    if t >= 1:
                        vector.wait_ge(sems["s_snd0"], 16 * t)
                    vector.tensor_copy(send_sb[:, 0:SL], psT0[:, 0:128]).then_inc(
                        sems["s_cp0"], 1
                    )
                if 2 <= t <= T + 1:
                    vector.wait_ge(sems["s_th1"], m2 + 1)
                    vector.tensor_tensor(
                        h1l_sb[:, :], g1_sb[:, 2 * SL:3 * SL], th1_sb[:, :], MUL
                    ).then_inc(sems["s_dve1"], 1)
                    # cp1(t): psT1 now holds h1(t-2); feeds AG(t)'s h1 half
                    vector.wait_ge(sems["s_t1"], t - 1)
                    vector.wait_ge(sems["s_snd1"], 16 * t)
                    vector.tensor_copy(send_sb[:, SL:2 * SL], psT1[:, 0:128]).then_inc(
                        sems["s_cp1"], 1
                    )
                # inline MLP: logits psum -> lg_sb
                for (g, d) in chunk_at.get(t, ()):
                    if d <= 7:
                        continue
                    i = d - 8
                    k = g % NC
                    og = g // NC
                    with vector.If_eq(rreg, k):
                        vector.wait_ge(sems["s_olog"], 4 * og + i + 1)
                        if og >= 1:
                            vector.wait_ge(sems["s_out"], 16 * (4 * (og - 1) + i + 1))
                        vector.tensor_copy(
                            lg_sb[:, i * VOCAB:(i + 1) * VOCAB], hs_ps[2][:, 0:VOCAB]
                        ).then_inc(sems["s_ocp"], 1)

        @block.gpsimd
        def _(gpsimd):
            rank = gpsimd.partition_id()
            rreg = gpsimd.to_reg(rank)
            ag_out3 = ag_out.rearrange("(j p) c -> p j c", p=128)
            for s in range(GEND):
                if s <= T + 1:
                    # sends (h1 half first: it is ready early in the step)
                    gpsimd.wait_ge(sems["s_cp1"], max(1, s))
                    if s >= 1:
                        gpsimd.wait_ge(sems["s_cc"], s)  # ag_in free
                    gpsimd.dma_start(
                        out=ag_in[:, SL:2 * SL], in_=send_sb[:, SL:2 * SL]
                    ).then_inc(sems["s_snd1"], 16)
                    if s <= T - 1:
                        gpsimd.wait_ge(sems["s_cp0"], s + 1)
                        gpsimd.dma_start(
                            out=ag_in[:, 0:SL], in_=send_sb[:, 0:SL]
                        ).then_inc(sems["s_snd0"], 16)
                    # doorbell
                    if s <= T - 1:
                        gpsimd.wait_ge(sems["s_snd0"], 16 * (s + 1))
                    gpsimd.wait_ge(sems["s_snd1"], 16 * (s + 1))
                    if s >= 3:
                        gpsimd.wait_ge(sems["s_mdma"], 16 * (s - 2))
                    gpsimd.collective_compute(
                        "AllGather",
                        mybir.AluOpType.bypass,
                        replica_groups=[list(range(NC))],
                        ins=[ag_in.ap().opt()],
                        outs=[ag_out.ap().opt()],
                    ).then_inc(sems["s_cc"], 1)
                    gpsimd.wait_ge(sems["s_cc"], s + 1)
                    # receives
                    p = (s % 2) * NN
                    if s <= T - 1:
                        if s >= 2:
                            gpsimd.wait_ge(sems["s_z1"], s - 1)  # WAR on h buffers
                        gpsimd.dma_start(
                            out=h0T_sb[:, p:p + NN].rearrange("p (j c) -> p j c", c=128),
                            in_=ag_out3[:, :, 0:SL],
                        ).then_inc(sems["s_rcv0"], 16)
                    if s <= T:
                        gpsimd.dma_start(
                            out=h1T_sb[:, p:p + NN].rearrange("p (j c) -> p j c", c=128),
                            in_=ag_out3[:, :, SL:2 * SL],
                        ).then_inc(sems["s_rcv1"], 16)
                    # mlp ring fill: h1(s-2) -> mlpbuf slot
                    if s >= 2:
                        u = s - 2
                        g = u // 4
                        i = u % 4
                        dst = mlpbuf[:, (g % MDEPTH) * q4:(g % MDEPTH + 1) * q4].rearrange(
                            "p (j i2 c) -> p j i2 c", i2=4, c=128
                        )[:, :, i, :]
                        gpsimd.dma_start(
                            out=dst, in_=ag_out3[:, :, SL:2 * SL]
                        ).then_inc(sems["s_mdma"], 16)
                # inline MLP: output DMAs
                for g in outdma_at.get(s, ()):
                    k = g % NC
                    og = g // NC
                    with gpsimd.If_eq(rreg, k):
                        gpsimd.wait_ge(sems["s_ocp"], 4 * (og + 1))
                        for i in range(4):
                            gpsimd.dma_start(
                                out=out[4 * og + i, :, :],
                                in_=lg_sb[:, i * VOCAB:(i + 1) * VOCAB],
                            ).then_inc(sems["s_out"], 16)

    return nc


def _host_prep(inputs, T):
    inp = np.ascontiguousarray(inputs["inputs"][:T]).astype(np.float32)
    emb_W = inputs["emb_W"].astype(np.float32)
    W0 = inputs["lstm_W0"].astype(np.float32)
    b0 = inputs["lstm_b0"].astype(np.float32)
    W1 = inputs["lstm_W1"].astype(np.float32)
    b1 = inputs["lstm_b1"].astype(np.float32)

    flat = inp.reshape(T * B, IND)
    s = np.where(
        (flat[:, VOCAB] == 1.0) & (flat[:, VOCAB + 1] == 0.0), 1.0, -1.0
    ).astype(np.float32)
    inpT_aug = np.zeros((KP, T * B), np.float32)
    inpT_aug[:IND] = flat.T
    inpT_aug[IND] = 1.0
    inpT_aug[IND + 1] = s

    # x-side folded weight: emb @ W0[:512] + flags(b0 row) + rank1(u row)
    Wc = np.zeros((KP, 4 * NN), np.float32)
    Wc[:IND] = emb_W @ W0[:EMB]
    Wc[IND] = b0
    Wc[IND + 1] = W0[EMB:EMB + BIG].sum(axis=0)

    W0h = W0[EMB + BIG:]            # [1024, 4096]
    W1x, W1h = W1[:NN], W1[NN:]

    def gate_cols(W, k):
        return np.concatenate(
            [W[:, base + k * SL:base + (k + 1) * SL] for base in
             (0, NN, 2 * NN, 3 * NN)], axis=1)

    bf = lambda x: np.ascontiguousarray(x).astype(BF16)
    inpT_bf = bf(inpT_aug)
    ow0 = bf(inputs["out_W0"])
    ob0c = np.ascontiguousarray(
        inputs["out_b0"].astype(np.float32).reshape(NC, 128).T
    )
    ow1 = bf(inputs["out_W1"])
    ob1r = bf(inputs["out_b1"].reshape(1, VOCAB))
    iden = bf(np.eye(128, dtype=np.float32))

    in_maps = []
    for k in range(NC):
        in_maps.append({
            "inpT": inpT_bf,
            "wc": bf(gate_cols(Wc, k)),
            "w0h": bf(gate_cols(W0h, k)),
            "w1x": bf(gate_cols(W1x, k)),
            "w1h": bf(gate_cols(W1h, k)),
            "b1r": bf(gate_cols(b1.reshape(1, 4 * NN), k)),
            "ow0": ow0,
            "ob0c": ob0c,
            "ow1": ow1,
            "ob1r": ob1r,
            "iden": iden,
        })
    return in_maps


_CACHE = {}


def run(inputs, T=T_FULL, trace=False):
    if T not in _CACHE:
        _CACHE[T] = build(T)
    nc = _CACHE[T]
    in_maps = _host_prep(inputs, T)
    res = run_bass_kernel_spmd(
        nc, in_maps, core_ids=list(range(NC)), trace=trace
    )
    # reassemble: group g (steps 4g..4g+3) was computed by core g%8 at
    # its local rows 4*(g//8)..4*(g//8)+3
    NG = T // 4
    out = np.empty((T, B, VOCAB), np.float32)
    for g in range(NG):
        k = g % NC
        og = g // NC
        out[4 * g:4 * g + 4] = res.results[k]["out"][4 * og:4 * og + 4]
    return out, res


def kernel(**inputs):
    out, _ = run(inputs, T=T_FULL)
    return out.astype(np.float32)


# revision 13
# speedup vs baseline: 1.1248x; 1.0133x over previous
"""Trainium2 Bass kernel for the 2-layer LSTM language model.

Strategy: 8-way tensor parallelism over the hidden/gate dimension.
- Core k owns hidden slice k (128 of 1024 units) of both LSTM layers:
  it computes gate columns [f_k|i_k|o_k|t_k] (512 of 4096) each step.
- Per step one AllGather distributes [h0T_k(t) | h1T_k(t-2)].  Layer 1
  lags layer 0 by TWO steps so the AllGather doorbell waits only on the
  (critical) layer-0 chain; the layer-1 half was transposed a full
  iteration earlier.  z1(m) runs at iteration m+2 and consumes
  h0T(m) / h1T(m-1) exactly as the math requires - only the wall-clock
  schedule shifts, not the values.
- The embedding, the speaker-flag rank-1 term, and b0 are folded into a
  single [384, 4096] input-side weight on the host; the x-part of z0
  accumulates into PSUM before the AllGather lands.
- The output MLP (relu(h1@ow0+b0)@ow1+b1) is computed INLINE: every
  step, the h1 tiles already distributed by the AllGather are DMA'd
  from ag_out into a deep SBUF ring (mlpbuf); groups of 4 steps are
  processed round-robin (group g -> core g%8) as small per-iteration
  chunks that fill tensor-engine idle time under the AllGather.  No
  h1 DRAM store, no trailing big AllGather, no output tail.
Matmul operands are bf16 (f32 PSUM accumulation); cell state stays f32.
"""
import numpy as np
import ml_dtypes

import concourse.bass as bass
import concourse.mybir as mybir
from concourse.bass_utils import run_bass_kernel_spmd

BF16 = ml_dtypes.bfloat16

T_FULL, B, IND = 512, 128, 259
EMB, NN, VOCAB, BIG = 512, 1024, 256, 128
NC = 8
SL = NN // NC          # 128 hidden units per core
GC = 4 * SL            # 512 gate columns per core
KP = 384               # padded inpT rows = 3 K-tiles (259 data + 1 + s + pad)
MDEPTH = 8             # mlpbuf ring depth in groups (8 groups = 32 steps)
AF = mybir.dt.ActivationFunctionType if hasattr(mybir.dt, "ActivationFunctionType") else mybir.ActivationFunctionType
BF = mybir.dt.bfloat16
F32 = mybir.dt.float32


def build(T):
    NG = T // 4            # output groups of 4 steps (round-robin over cores)
    TG = 4 * (NG // NC)    # output rows per core
    assert NG % NC == 0
    nc = bass.Bass(target_bir_lowering=False, num_devices=NC)

    # ---- DRAM parameters (per core) ----
    inpT = nc.declare_dram_parameter("inpT", [KP, T * B], BF, isOutput=False)
    wc = nc.declare_dram_parameter("wc", [KP, GC], BF, isOutput=False)
    w0h = nc.declare_dram_parameter("w0h", [NN, GC], BF, isOutput=False)
    w1x = nc.declare_dram_parameter("w1x", [NN, GC], BF, isOutput=False)
    w1h = nc.declare_dram_parameter("w1h", [NN, GC], BF, isOutput=False)
    b1r = nc.declare_dram_parameter("b1r", [1, GC], BF, isOutput=False)
    ow0 = nc.declare_dram_parameter("ow0", [NN, NN], BF, isOutput=False)
    ob0c = nc.declare_dram_parameter("ob0c", [128, NC], F32, isOutput=False)
    ow1 = nc.declare_dram_parameter("ow1", [NN, VOCAB], BF, isOutput=False)
    ob1r = nc.declare_dram_parameter("ob1r", [1, VOCAB], BF, isOutput=False)
    iden = nc.declare_dram_parameter("iden", [128, 128], BF, isOutput=False)
    out = nc.declare_dram_parameter("out", [TG, B, VOCAB], F32, isOutput=True)

    # ---- internal DRAM ----
    ag_in = nc.dram_tensor("ag_in", [128, 2 * SL], BF)
    ag_out = nc.dram_tensor("ag_out", [NC * 128, 2 * SL], BF, addr_space="Shared")

    # ---- SBUF ----
    wc_sb = nc.alloc_sbuf_tensor("wc_sb", [128, 3 * GC], BF)
    w0h_sb = nc.alloc_sbuf_tensor("w0h_sb", [128, 8 * GC], BF)
    w1x_sb = nc.alloc_sbuf_tensor("w1x_sb", [128, 8 * GC], BF)
    w1h_sb = nc.alloc_sbuf_tensor("w1h_sb", [128, 8 * GC], BF)
    b1_sb = nc.alloc_sbuf_tensor("b1_sb", [1, GC], BF)
    ones_sb = nc.alloc_sbuf_tensor("ones_sb", [1, 128], BF)
    id_sb = nc.alloc_sbuf_tensor("id_sb", [128, 128], BF)
    xb_sb = nc.alloc_sbuf_tensor("xb_sb", [128, 4 * 3 * 128], BF)  # 4 bufs x 3 tiles
    h0T_sb = nc.alloc_sbuf_tensor("h0T_sb", [128, 2 * NN], BF)  # 2 bufs x 8 tiles
    h1T_sb = nc.alloc_sbuf_tensor("h1T_sb", [128, 2 * NN], BF)
    g0_sb = nc.alloc_sbuf_tensor("g0_sb", [128, 3 * SL], F32)
    t0_sb = nc.alloc_sbuf_tensor("t0_sb", [128, SL], F32)
    th0_sb = nc.alloc_sbuf_tensor("th0_sb", [128, SL], F32)
    g1_sb = nc.alloc_sbuf_tensor("g1_sb", [128, 3 * SL], F32)
    t1_sb = nc.alloc_sbuf_tensor("t1_sb", [128, SL], F32)
    th1_sb = nc.alloc_sbuf_tensor("th1_sb", [128, SL], F32)
    c0_sb = nc.alloc_sbuf_tensor("c0_sb", [128, SL], F32)
    c1_sb = nc.alloc_sbuf_tensor("c1_sb", [128, SL], F32)
    tmpa_sb = nc.alloc_sbuf_tensor("tmpa_sb", [128, SL], F32)
    tmpb_sb = nc.alloc_sbuf_tensor("tmpb_sb", [128, SL], F32)
    h0l_sb = nc.alloc_sbuf_tensor("h0l_sb", [128, SL], BF)
    h1l_sb = nc.alloc_sbuf_tensor("h1l_sb", [128, SL], BF)
    send_sb = nc.alloc_sbuf_tensor("send_sb", [128, 2 * SL], BF)
    # inline output MLP
    mlpbuf = nc.alloc_sbuf_tensor("mlpbuf", [128, MDEPTH * 8 * 4 * 128], BF)
    ow0_sb = nc.alloc_sbuf_tensor("ow0_sb", [128, 8 * NN], BF)
    ow1_sb = nc.alloc_sbuf_tensor("ow1_sb", [128, 8 * VOCAB], BF)
    ob0_sb = nc.alloc_sbuf_tensor("ob0_sb", [128, NC], F32)
    ob1_sb = nc.alloc_sbuf_tensor("ob1_sb", [1, VOCAB], BF)
    hsT_sb = nc.alloc_sbuf_tensor("hsT_sb", [128, 8 * 512], BF)
    lg_sb = nc.alloc_sbuf_tensor("lg_sb", [128, 4 * VOCAB], F32)

    # ---- PSUM (7 of 8 banks) ----
    psA = nc.alloc_psum_tensor("psA", [128, 512], F32)
    psB = nc.alloc_psum_tensor("psB", [128, 512], F32)
    psT0 = nc.alloc_psum_tensor("psT0", [128, 128], BF)
    psT1 = nc.alloc_psum_tensor("psT1", [128, 128], BF)
    hs_ps = [nc.alloc_psum_tensor(f"hs_ps{i}", [128, 512], F32) for i in range(3)]

    # ---- semaphores ----
    sems = {}
    for name in (
        "s_init", "s_vinit", "s_x", "s_xdone", "s_z0", "s_z1", "s_act0",
        "s_act1", "s_cmid0", "s_cmid1", "s_th0", "s_th1", "s_dve0", "s_dve1",
        "s_t0", "s_t1", "s_cp0", "s_cp1", "s_snd0", "s_rcv0",
        "s_rcv1", "s_cc", "s_mcp",
        "s_ohsT", "s_orelu", "s_olog", "s_ocp", "s_out",
    ):
        sems[name] = nc.alloc_semaphore(name)

    # ---- inline-MLP chunk schedule ----
    # group g covers steps 4g..4g+3, owner core g%8, data complete after
    # gpsimd step 4g+5 (s_mdma >= 16*(4g+4)).  Chunks at iteration
    # tau = 4g+6+d: d=0..7 -> hs m-tile d; d=8..11 -> logits step d-8.
    # Owner-local group ordinal og = g//8.
    chunk_at = {}
    for g in range(NG):
        for d in range(12):
            chunk_at.setdefault(4 * g + 6 + d, []).append((g, d))
    outdma_at = {}
    for g in range(NG):
        outdma_at.setdefault(4 * g + 18, []).append(g)
    TEND = max(T + 2, max(chunk_at) + 1)
    GEND = max(T + 2, max(outdma_at) + 1)

    q4 = 8 * 4 * 128  # mlpbuf columns per group slot

    with nc.Block() as block:

        @block.sync
        def _(sync):
            n_init = 0

            def ld(dst, src):
                nonlocal n_init
                sync.dma_start(out=dst, in_=src).then_inc(sems["s_init"], 16)
                n_init += 1

            for j in range(3):
                ld(wc_sb[:, j * GC:(j + 1) * GC], wc[j * 128:(j + 1) * 128, :])
            for j in range(8):
                ld(w0h_sb[:, j * GC:(j + 1) * GC], w0h[j * 128:(j + 1) * 128, :])
                ld(w1x_sb[:, j * GC:(j + 1) * GC], w1x[j * 128:(j + 1) * 128, :])
                ld(w1h_sb[:, j * GC:(j + 1) * GC], w1h[j * 128:(j + 1) * 128, :])
                ld(ow0_sb[:, j * NN:(j + 1) * NN], ow0[j * 128:(j + 1) * 128, :])
                ld(ow1_sb[:, j * VOCAB:(j + 1) * VOCAB], ow1[j * 128:(j + 1) * 128, :])
            ld(b1_sb[:, :], b1r[:, :])
            ld(id_sb[:, :], iden[:, :])
            ld(ob0_sb[:, :], ob0c[:, :])
            ld(ob1_sb[:, :], ob1r[:, :])
            assert n_init == 47, n_init
            # x prefetch: inpT column block t -> xb slot t%4 (3 k-tiles)
            inpT3 = inpT.rearrange("(j p) n -> p j n", p=128)
            for t in range(T):
                if t >= 4:
                    sync.wait_ge(sems["s_xdone"], t - 3)
                s = (t % 4) * 3 * 128
                dst = xb_sb[:, s:s + 3 * 128].rearrange("p (j c) -> p j c", c=128)
                sync.dma_start(
                    out=dst, in_=inpT3[:, :, t * B:(t + 1) * B]
                ).then_inc(sems["s_x"], 16)

        @block.tensor
        def _(tensor):
            tensor.wait_ge(sems["s_init"], 16 * 47)
            tensor.wait_ge(sems["s_vinit"], 1)
            rreg = tensor.to_reg(tensor.partition_id())
            for t in range(TEND):
                m2 = t - 2
                if 2 <= t <= T + 1:
                    # z1(m2) bias + w1x part first: h0T(m2) came from an
                    # older AllGather, so this fills the current AG wait.
                    if m2 >= 1:
                        tensor.wait_ge(sems["s_act1"], m2)
                    if t >= T:
                        tensor.wait_ge(sems["s_rcv0"], 16 * T)
                    tensor.matmul(psB[:, :], ones_sb[:, :], b1_sb[:, :], start=True, stop=False)
                    p0 = (m2 % 2) * NN
                    for j in range(8):
                        tensor.matmul(
                            psB[:, :],
                            h0T_sb[:, p0 + j * 128:p0 + (j + 1) * 128],
                            w1x_sb[:, j * GC:(j + 1) * GC],
                            start=False, stop=False,
                        )
                if t <= T - 1:
                    # z0(t) x-part into psA (before the AllGather lands)
                    tensor.wait_ge(sems["s_x"], 16 * (t + 1))
                    if t >= 1:
                        tensor.wait_ge(sems["s_act0"], t)
                    xoff = (t % 4) * 3 * 128
                    for j in range(3):
                        ins = tensor.matmul(
                            psA[:, :],
                            xb_sb[:, xoff + j * 128:xoff + (j + 1) * 128],
                            wc_sb[:, j * GC:(j + 1) * GC],
                            start=(j == 0),
                            stop=(t == 0 and j == 2),
                        )
                        if j == 2:
                            ins.then_inc(sems["s_xdone"], 1)
                    # z0(t) h-part
                    if t >= 1:
                        tensor.wait_ge(sems["s_rcv0"], 16 * t)
                        p = ((t - 1) % 2) * NN
                        for j in range(8):
                            ins = tensor.matmul(
                                psA[:, :],
                                h0T_sb[:, p + j * 128:p + (j + 1) * 128],
                                w0h_sb[:, j * GC:(j + 1) * GC],
                                start=False,
                                stop=(j == 7),
                            )
                            if j == 7:
                                ins.then_inc(sems["s_z0"], 1)
                if 2 <= t <= T + 1:
                    # z1(m2) w1h part: h1T(m2-1) arrives with the same AG as
                    # h0T(t-1), right before this point.
                    tensor.wait_ge(sems["s_rcv1"], 16 * t)
                    p1 = ((m2 + 1) % 2) * NN
                    for j in range(8):
                        ins = tensor.matmul(
                            psB[:, :],
                            h1T_sb[:, p1 + j * 128:p1 + (j + 1) * 128],
                            w1h_sb[:, j * GC:(j + 1) * GC],
                            start=False, stop=(j == 7),
                        )
                        if j == 7:
                            ins.then_inc(sems["s_z1"], 1)
                if t <= T - 1:
                    # transpose h0l(t) -> psT0; WAR: cp0(t-1) read psT0
                    tensor.wait_ge(sems["s_dve0"], t + 1)
                    if t >= 1:
                        tensor.wait_ge(sems["s_cp0"], t)
                    tensor.transpose(psT0[:, 0:128], h0l_sb[:, :], id_sb[:, :]).then_inc(
                        sems["s_t0"], 1
                    )
                if 2 <= t <= T + 1:
                    # transpose h1l(m2) -> psT1; WAR: cp1(t-1) read psT1
                    tensor.wait_ge(sems["s_dve1"], t - 1)
                    if t >= 3:
                        tensor.wait_ge(sems["s_cp1"], t - 1)
                    tensor.transpose(psT1[:, 0:128], h1l_sb[:, :], id_sb[:, :]).then_inc(
                        sems["s_t1"], 1
                    )
                # ---- inline MLP chunks ----
                for (g, d) in chunk_at.get(t, ()):
                    k = g % NC
                    og = g // NC
                    with tensor.If_eq(rreg, k):
                        if d == 0:
                            tensor.wait_ge(sems["s_mcp"], 16 * (4 * g + 4))
                        if d <= 7:
                            # hs m-tile d -> hs_ps[d%2]
                            if og * 8 + d >= 2:
                                tensor.wait_ge(sems["s_orelu"], 8 * og + d - 1)
                            mb = (g % MDEPTH) * q4
                            for j in range(8):
                                ins = tensor.matmul(
                                    hs_ps[d % 2][:, :],
                                    ow0_sb[:, j * NN + d * 128:j * NN + (d + 1) * 128],
                                    mlpbuf[:, mb + j * 512:mb + (j + 1) * 512],
                                    start=(j == 0), stop=(j == 7),
                                )
                                if j == 7:
                                    ins.then_inc(sems["s_ohsT"], 1)
                        else:
                            i = d - 8
                            # logits for step 4g+i -> hs_ps[2]
                            if d == 8:
                                tensor.wait_ge(sems["s_orelu"], 8 * (og + 1))
                            if og * 4 + i >= 1:
                                tensor.wait_ge(sems["s_ocp"], 4 * og + i)
                            tensor.matmul(
                                hs_ps[2][:, 0:VOCAB], ones_sb[:, :], ob1_sb[:, :],
                                start=True, stop=False,
                            )
                            for mm in range(8):
                                ins = tensor.matmul(
                                    hs_ps[2][:, 0:VOCAB],
                                    hsT_sb[:, mm * 512 + i * 128:mm * 512 + (i + 1) * 128],
                                    ow1_sb[:, mm * VOCAB:(mm + 1) * VOCAB],
                                    start=False, stop=(mm == 7),
                                )
                                if mm == 7:
                                    ins.then_inc(sems["s_olog"], 1)

        @block.scalar
        def _(scalar):
            scalar.wait_ge(sems["s_init"], 16 * 47)
            rreg = scalar.to_reg(scalar.partition_id())
            SIG = AF.Sigmoid
            TANH = AF.Tanh
            for t in range(TEND):
                m2 = t - 2
                if t <= T - 1:
                    if t == 0:
                        scalar.wait_ge(sems["s_xdone"], 1)
                    else:
                        scalar.wait_ge(sems["s_z0"], t)
                    if t >= 1:
                        scalar.wait_ge(sems["s_cmid0"], t)  # g0/t0 WAR
                    scalar.activation(g0_sb[:, :], psA[:, 0:3 * SL], SIG)
                    scalar.activation(t0_sb[:, :], psA[:, 3 * SL:4 * SL], TANH).then_inc(
                        sems["s_act0"], 1
                    )
                if 2 <= t <= T + 1:
                    scalar.wait_ge(sems["s_z1"], m2 + 1)
                    if m2 >= 1:
                        scalar.wait_ge(sems["s_cmid1"], m2)
                    scalar.activation(g1_sb[:, :], psB[:, 0:3 * SL], SIG)
                    scalar.activation(t1_sb[:, :], psB[:, 3 * SL:4 * SL], TANH).then_inc(
                        sems["s_act1"], 1
                    )
                if t <= T - 1:
                    scalar.wait_ge(sems["s_cmid0"], t + 1)
                    if t >= 1:
                        scalar.wait_ge(sems["s_dve0"], t)  # th0 WAR
                    scalar.activation(th0_sb[:, :], c0_sb[:, :], TANH).then_inc(
                        sems["s_th0"], 1
                    )
                if 2 <= t <= T + 1:
                    scalar.wait_ge(sems["s_cmid1"], m2 + 1)
                    if m2 >= 1:
                        scalar.wait_ge(sems["s_dve1"], m2)
                    scalar.activation(th1_sb[:, :], c1_sb[:, :], TANH).then_inc(
                        sems["s_th1"], 1
                    )
                # inline MLP: relu for hs m-tiles finished at this iteration
                for (g, d) in chunk_at.get(t, ()):
                    if d > 7:
                        continue
                    k = g % NC
                    og = g // NC
                    with scalar.If_eq(rreg, k):
                        scalar.wait_ge(sems["s_ohsT"], 8 * og + d + 1)
                        if og >= 1 and d <= 1:
                            scalar.wait_ge(sems["s_olog"], 4 * og)
                        scalar.activation(
                            hsT_sb[:, d * 512:(d + 1) * 512],
                            hs_ps[d % 2][:, :],
                            AF.Relu,
                            bias=ob0_sb[:, d:d + 1],
                        ).then_inc(sems["s_orelu"], 1)

        @block.vector
        def _(vector):
            vector.memset(send_sb[:, :], 0.0).then_inc(sems["s_cp1"], 1)
            vector.memset(c0_sb[:, :], 0.0)
            vector.memset(c1_sb[:, :], 0.0)
            vector.memset(ones_sb[:, :], 1.0).then_inc(sems["s_vinit"], 1)
            rreg = vector.to_reg(vector.partition_id())
            MUL = mybir.AluOpType.mult
            for t in range(TEND):
                m2 = t - 2
                if t <= T - 1:
                    vector.wait_ge(sems["s_act0"], t + 1)
                    vector.tensor_tensor(tmpa_sb[:, :], g0_sb[:, 0:SL], c0_sb[:, :], MUL)
                    vector.tensor_tensor(tmpb_sb[:, :], g0_sb[:, SL:2 * SL], t0_sb[:, :], MUL)
                    vector.tensor_add(c0_sb[:, :], tmpa_sb[:, :], tmpb_sb[:, :]).then_inc(
                        sems["s_cmid0"], 1
                    )
                if 2 <= t <= T + 1:
                    vector.wait_ge(sems["s_act1"], m2 + 1)
                    vector.tensor_tensor(tmpa_sb[:, :], g1_sb[:, 0:SL], c1_sb[:, :], MUL)
                    vector.tensor_tensor(tmpb_sb[:, :], g1_sb[:, SL:2 * SL], t1_sb[:, :], MUL)
                    vector.tensor_add(c1_sb[:, :], tmpa_sb[:, :], tmpb_sb[:, :]).then_inc(
                        sems["s_cmid1"], 1
                    )
                if t <= T - 1:
                    vector.wait_ge(sems["s_th0"], t + 1)
                    vector.tensor_tensor(
                        h0l_sb[:, :], g0_sb[:, 2 * SL:3 * SL], th0_sb[:, :], MUL
                    ).then_inc(sems["s_dve0"], 1)
                    vector.wait_ge(sems["s_t0"], t + 1)
                    if t >= 1:
                        vector.wait_ge(sems["s_snd0"], 16 * t)
                    vector.tensor_copy(send_sb[:, 0:SL], psT0[:, 0:128]).then_inc(
                        sems["s_cp0"], 1
                    )
                if 2 <= t <= T + 1:
                    vector.wait_ge(sems["s_th1"], m2 + 1)
                    vector.tensor_tensor(
                        h1l_sb[:, :], g1_sb[:, 2 * SL:3 * SL], th1_sb[:, :], MUL
                    ).then_inc(sems["s_dve1"], 1)
                    # cp1(t): psT1 now holds h1(t-2); feeds AG(t)'s h1 half
                    vector.wait_ge(sems["s_t1"], t - 1)
                    vector.wait_ge(sems["s_snd0"], 16 * t)
                    vector.tensor_copy(send_sb[:, SL:2 * SL], psT1[:, 0:128]).then_inc(
                        sems["s_cp1"], 1
                    )
                # inline MLP: logits psum -> lg_sb
                for (g, d) in chunk_at.get(t, ()):
                    if d <= 7:
                        continue
                    i = d - 8
                    k = g % NC
                    og = g // NC
                    with vector.If_eq(rreg, k):
                        vector.wait_ge(sems["s_olog"], 4 * og + i + 1)
                        if og >= 1:
                            vector.wait_ge(sems["s_out"], 16 * (4 * (og - 1) + i + 1))
                        vector.tensor_copy(
                            lg_sb[:, i * VOCAB:(i + 1) * VOCAB], hs_ps[2][:, 0:VOCAB]
                        ).then_inc(sems["s_ocp"], 1)

        @block.gpsimd
        def _(gpsimd):
            rank = gpsimd.partition_id()
            rreg = gpsimd.to_reg(rank)
            ag_out3 = ag_out.rearrange("(j p) c -> p j c", p=128)
            for s in range(GEND):
                if s <= T + 1:
                    # single combined send of both halves
                    gpsimd.wait_ge(sems["s_cp1"], max(1, s))
                    if s <= T - 1:
                        gpsimd.wait_ge(sems["s_cp0"], s + 1)
                    if s >= 1:
                        gpsimd.wait_ge(sems["s_cc"], s)  # ag_in free
                    if s <= T - 1:
                        gpsimd.dma_start(
                            out=ag_in[:, :], in_=send_sb[:, :]
                        ).then_inc(sems["s_snd0"], 16)
                    else:
                        gpsimd.dma_start(
                            out=ag_in[:, SL:2 * SL], in_=send_sb[:, SL:2 * SL]
                        ).then_inc(sems["s_snd0"], 16)
                    # doorbell
                    gpsimd.wait_ge(sems["s_snd0"], 16 * (s + 1))
                    if s >= 3:
                        gpsimd.wait_ge(sems["s_mcp"], 16 * (s - 2))
                    gpsimd.collective_compute(
                        "AllGather",
                        mybir.AluOpType.bypass,
                        replica_groups=[list(range(NC))],
                        ins=[ag_in.ap().opt()],
                        outs=[ag_out.ap().opt()],
                    ).then_inc(sems["s_cc"], 1)
                    gpsimd.wait_ge(sems["s_cc"], s + 1)
                    # receives
                    p = (s % 2) * NN
                    if s <= T - 1:
                        if s >= 2:
                            gpsimd.wait_ge(sems["s_z1"], s - 1)  # WAR on h buffers
                        gpsimd.dma_start(
                            out=h0T_sb[:, p:p + NN].rearrange("p (j c) -> p j c", c=128),
                            in_=ag_out3[:, :, 0:SL],
                        ).then_inc(sems["s_rcv0"], 16)
                    if s <= T + 1:
                        gpsimd.dma_start(
                            out=h1T_sb[:, p:p + NN].rearrange("p (j c) -> p j c", c=128),
                            in_=ag_out3[:, :, SL:2 * SL],
                        ).then_inc(sems["s_rcv1"], 16)
                    # mlp ring fill: h1(s-2) -> mlpbuf slot
                    if s >= 2:
                        u = s - 2
                        g = u // 4
                        i = u % 4
                        dst = mlpbuf[:, (g % MDEPTH) * q4:(g % MDEPTH + 1) * q4].rearrange(
                            "p (j i2 c) -> p j i2 c", i2=4, c=128
                        )[:, :, i, :]
                        gpsimd.dma_start(
                            out=dst, in_=ag_out3[:, :, SL:2 * SL]
                        ).then_inc(sems["s_mcp"], 16)
                # inline MLP: output DMAs
                for g in outdma_at.get(s, ()):
                    k = g % NC
                    og = g // NC
                    with gpsimd.If_eq(rreg, k):
                        gpsimd.wait_ge(sems["s_ocp"], 4 * (og + 1))
                        for i in range(4):
                            gpsimd.dma_start(
                                out=out[4 * og + i, :, :],
                                in_=lg_sb[:, i * VOCAB:(i + 1) * VOCAB],
                            ).then_inc(sems["s_out"], 16)

    from concourse.bacc import Bacc

    Bacc.insert_act_table_loads(nc)
    return nc


def _host_prep(inputs, T):
    inp = np.ascontiguousarray(inputs["inputs"][:T]).astype(np.float32)
    emb_W = inputs["emb_W"].astype(np.float32)
    W0 = inputs["lstm_W0"].astype(np.float32)
    b0 = inputs["lstm_b0"].astype(np.float32)
    W1 = inputs["lstm_W1"].astype(np.float32)
    b1 = inputs["lstm_b1"].astype(np.float32)

    flat = inp.reshape(T * B, IND)
    s = np.where(
        (flat[:, VOCAB] == 1.0) & (flat[:, VOCAB + 1] == 0.0), 1.0, -1.0
    ).astype(np.float32)
    inpT_aug = np.zeros((KP, T * B), np.float32)
    inpT_aug[:IND] = flat.T
    inpT_aug[IND] = 1.0
    inpT_aug[IND + 1] = s

    # x-side folded weight: emb @ W0[:512] + flags(b0 row) + rank1(u row)
    Wc = np.zeros((KP, 4 * NN), np.float32)
    Wc[:IND] = emb_W @ W0[:EMB]
    Wc[IND] = b0
    Wc[IND + 1] = W0[EMB:EMB + BIG].sum(axis=0)

    W0h = W0[EMB + BIG:]            # [1024, 4096]
    W1x, W1h = W1[:NN], W1[NN:]

    def gate_cols(W, k):
        return np.concatenate(
            [W[:, base + k * SL:base + (k + 1) * SL] for base in
             (0, NN, 2 * NN, 3 * NN)], axis=1)

    bf = lambda x: np.ascontiguousarray(x).astype(BF16)
    inpT_bf = bf(inpT_aug)
    ow0 = bf(inputs["out_W0"])
    ob0c = np.ascontiguousarray(
        inputs["out_b0"].astype(np.float32).reshape(NC, 128).T
    )
    ow1 = bf(inputs["out_W1"])
    ob1r = bf(inputs["out_b1"].reshape(1, VOCAB))
    iden = bf(np.eye(128, dtype=np.float32))

    in_maps = []
    for k in range(NC):
        in_maps.append({
            "inpT": inpT_bf,
            "wc": bf(gate_cols(Wc, k)),
            "w0h": bf(gate_cols(W0h, k)),
            "w1x": bf(gate_cols(W1x, k)),
            "w1h": bf(gate_cols(W1h, k)),
            "b1r": bf(gate_cols(b1.reshape(1, 4 * NN), k)),
            "ow0": ow0,
            "ob0c": ob0c,
            "ow1": ow1,
            "ob1r": ob1r,
            "iden": iden,
        })
    return in_maps


_CACHE = {}


def run(inputs, T=T_FULL, trace=False):
    if T not in _CACHE:
        _CACHE[T] = build(T)
    nc = _CACHE[T]
    in_maps = _host_prep(inputs, T)
    res = run_bass_kernel_spmd(
        nc, in_maps, core_ids=list(range(NC)), trace=trace
    )
    # reassemble: group g (steps 4g..4g+3) was computed by core g%8 at
    # its local rows 4*(g//8)..4*(g//8)+3
    NG = T // 4
    out = np.empty((T, B, VOCAB), np.float32)
    for g in range(NG):
        k = g % NC
        og = g // NC
        out[4 * g:4 * g + 4] = res.results[k]["out"][4 * og:4 * og + 4]
    return out, res


def kernel(**inputs):
    out, _ = run(inputs, T=T_FULL)
    return out.astype(np.float32)
